# revision 10
# baseline (speedup 1.0000x reference)
"""AdaptiveGCN kernel for TRN2 (8 NeuronCores, SPMD).

Reference math (B=4, D=128, N=512):
    A = W1 @ x[b]                  # [D, N]
    C = W2 @ x[b] + b[:, None]     # [D, N]
    pre[d, i, j] = A[d, j] + (C - A)[d, i]
    out[d, i] = max_j relu(pre[d, i, j])

Since (C - A)[d, i] is constant in j and relu/max commute (both monotone),
    out[d, i] = relu(max_j (A[d, j] + b[d]) + ((W2 - W1) @ x[b])[d, i])
The [N, N] pairwise grid never materializes.

Sharding: one batch per core (cores 4..7 duplicate batches 0..3 and are
ignored on gather) — no cross-core communication needed.

Perf notes:
- Each dma_start costs ~0.6us of serialized sequencer issue (DIRECT2D), so
  inputs ship as ONE [128, 896] bf16 blob: x | W1^T | (W2-W1)^T | b_row,
  with b replicated into blob row 0 cols 768:896 as a [1, 128] row.
  896 cols = 1792 B/partition (multiple of 128 B) avoids HWDGE stragglers.
- b is folded into A via a rank-1 accumulate: a K=1 matmul with lhsT =
  b_row [1, 128] and rhs = ones [1, 512] (memset tile), so max_j gives
  amax + b directly and the DVE chain is just MAX -> (V+tvec) relu.
- bf16 compute/out (host pre-cast, pre-transposed weights); rel-err ~2e-3
  vs the 2e-2 gate. Output upcast to f32 on the host.
"""

import numpy as np
import ml_dtypes

import concourse.bacc as bacc
import concourse.tile as tile
from concourse import mybir
from concourse.bass_utils import run_bass_kernel_spmd

F32 = mybir.dt.float32
BF16 = mybir.dt.bfloat16
B, D, N = 4, 128, 512
BLOB_W = N + 3 * D  # 896
N_CORES = 8

_NC_CACHE = None


def _build():
    nc = bacc.Bacc(
        "TRN2", target_bir_lowering=False, debug=False, num_devices=N_CORES
    )
    blob = nc.declare_dram_parameter("blob", [D, BLOB_W], BF16, isOutput=False)
    out = nc.declare_dram_parameter("out", [D, N], BF16, isOutput=True)

    with tile.TileContext(nc) as tc:
        with (
            tc.tile_pool(name="sb", bufs=1) as sb,
            tc.tile_pool(name="ps", bufs=1, space="PSUM") as ps,
        ):
            ones_t = sb.tile([1, N], BF16)
            nc.gpsimd.memset(ones_t, 1.0)

            blob_t = sb.tile([D, BLOB_W], BF16)
            nc.sync.dma_start(out=blob_t, in_=blob[:, :])
            x_v = blob_t[:, 0:N]
            w1T_v = blob_t[:, N : N + D]
            wdT_v = blob_t[:, N + D : N + 2 * D]
            brow_v = blob_t[0:1, N + 2 * D : N + 3 * D]

            # A + b = W1 @ x + b 1^T  -> [D, N] f32 in PSUM (one bank)
            p_a = ps.tile([D, N], F32)
            nc.tensor.matmul(p_a, w1T_v, x_v, start=True, stop=False)
            nc.tensor.matmul(p_a, brow_v, ones_t, start=False, stop=True)

            # V = (W2 - W1) @ x -> [D, N]
            p_v = ps.tile([D, N], F32)
            nc.tensor.matmul(p_v, wdT_v, x_v, start=True, stop=True)

            # tvec[d] = max_j (Aforward[d, j] + b[d])
            tvec = sb.tile([D, 1], F32)
            nc.vector.reduce_max(out=tvec, in_=p_a, axis=mybir.AxisListType.X)

            # out = relu(V + tvec) = (V + tvec) max 0, fused on DVE
            o_t = sb.tile([D, N], BF16)
            nc.vector.tensor_scalar(
                out=o_t,
                in0=p_v,
                scalar1=tvec,
                scalar2=0.0,
                op0=mybir.AluOpType.add,
                op1=mybir.AluOpType.max,
            )
            nc.sync.dma_start(out=out[:, :], in_=o_t)
    nc.finalize()
    return nc


def _in_maps(x, W1, W2, b):
    bf = ml_dtypes.bfloat16
    x = np.asarray(x, dtype=np.float32)
    W1 = np.asarray(W1, dtype=np.float32)
    W2 = np.asarray(W2, dtype=np.float32)
    b = np.asarray(b, dtype=np.float32)
    brow = np.zeros((D, D), dtype=np.float32)
    brow[0, :] = b
    blobs = [
        np.ascontiguousarray(
            np.concatenate([x[c % B], W1.T, (W2 - W1).T, brow], axis=1)
        ).astype(bf)
        for c in range(N_CORES)
    ]
    return [{"blob": blobs[c]} for c in range(N_CORES)]


def kernel_raw(x, W1, W2, b, **run_kwargs):
    """Run the SPMD kernel; returns (full_output, BassKernelResults)."""
    global _NC_CACHE
    if _NC_CACHE is None:
        _NC_CACHE = _build()
    res = run_bass_kernel_spmd(
        _NC_CACHE, _in_maps(x, W1, W2, b), core_ids=list(range(N_CORES)),
        **run_kwargs,
    )
    out = np.stack(
        [res.results[c]["out"].astype(np.float32) for c in range(B)], axis=0
    )
    return out, res


def kernel(x, W1, W2, b):
    return kernel_raw(x, W1, W2, b)[0]


# revision 21
# speedup vs baseline: 1.3180x; 1.3180x over previous
"""AdaptiveGCN kernel for TRN2 (8 NeuronCores, SPMD).

Reference math (B=4, D=128, N=512):
    A = W1 @ x[b]                  # [D, N]
    C = W2 @ x[b] + b[:, None]     # [D, N]
    pre[d, i, j] = A[d, j] + (C - A)[d, i]
    out[d, i] = max_j relu(pre[d, i, j])

Since (C - A)[d, i] is constant in j and relu/max commute (both monotone),
    out[d, i] = relu(max_j A[d, j] + V[d, i] + b[d]),  V = (W2 - W1) @ x[b]
and with the further identity max(z + b, 0) = max(z, -b) + b the device
only computes q[d, i] = max(V[d, i] + amax[d], -b[d]); the final +b runs
on the host during the f32 upcast. The [N, N] pairwise grid never
materializes.

Sharding: one batch per core (cores 4..7 duplicate batches 0..3 and are
ignored on gather) — no cross-core communication needed.

Implementation: raw bacc blocks (no TileContext) — the dataflow is a
simple DMA -> PE -> DVE -> DMA chain with every cross-engine dependency
an explicit semaphore starting from 0, so the Bass-preamble and
Block-end all-engine barriers are skipped (engines still get per-engine
drains via the no_gpsimd_drain path).

Perf notes:
- Each dma_start costs ~0.6us of sequencer issue (DIRECT2D) plus ~0.7us
  doorbell-to-data latency, so the two input loads are issued
  concurrently by different HWDGE engines (sync: x, scalar: weights).
- No completion wait after the output DMA: NRT quiesces the DMA rings
  before results are readable (verified by writing 4MB with no wait —
  always correct), saving the ~1.4us completion-semaphore latency.
- bf16 compute/out (host pre-cast, pre-transposed weights); rel-err
  ~2e-3 vs the 2e-2 gate; output upcast to f32 (+b) on the host.
"""

from contextlib import ExitStack

import numpy as np
import ml_dtypes

import concourse.bass as bass_mod
import concourse.bacc as bacc
from concourse import mybir
from concourse.bass_utils import run_bass_kernel_spmd

F32 = mybir.dt.float32
BF16 = mybir.dt.bfloat16
B, D, N = 4, 128, 512
WB_W = 3 * D  # 384: w1T | wdT | -b | zero-pad
N_CORES = 8

_NC_CACHE = None


def _build():
    # Skip the Bass-preamble and Block-end all-engine barriers: every
    # cross-engine dep below is an explicit semaphore starting from 0.
    orig_barrier = bass_mod.Bass.all_engine_barrier
    bass_mod.Bass.all_engine_barrier = lambda self, **kw: None
    try:
        nc = bacc.Bacc(
            "TRN2", target_bir_lowering=False, debug=False,
            num_devices=N_CORES,
        )
        xb = nc.declare_dram_parameter("xb", [D, N], BF16, isOutput=False)
        wb = nc.declare_dram_parameter("wb", [D, WB_W], BF16, isOutput=False)
        out = nc.declare_dram_parameter("out", [D, N], BF16, isOutput=True)

        with ExitStack() as ctx:
            x_t = ctx.enter_context(nc.sbuf_tensor("x_t", [D, N], BF16))
            wb_t = ctx.enter_context(nc.sbuf_tensor("wb_t", [D, WB_W], BF16))
            o_t = ctx.enter_context(nc.sbuf_tensor("o_t", [D, N], BF16))
            amax = ctx.enter_context(nc.sbuf_tensor("amax", [D, 1], F32))
            negb = ctx.enter_context(nc.sbuf_tensor("negb", [D, 1], F32))
            p_a = ctx.enter_context(nc.psum_tensor("p_a", [D, N], F32))
            p_v = ctx.enter_context(nc.psum_tensor("p_v", [D, N], F32))
            dma_a = ctx.enter_context(nc.semaphore("dma_a"))
            dma_b = ctx.enter_context(nc.semaphore("dma_b"))
            pe_sem = ctx.enter_context(nc.semaphore("pe_sem"))
            dve_sem = ctx.enter_context(nc.semaphore("dve_sem"))

            w1T_v = wb_t[:, 0:D]
            wdT_v = wb_t[:, D : 2 * D]
            negb_v = wb_t[:, 2 * D : 2 * D + 1]

            with nc.Block(no_gpsimd_drain=True) as block:

                @block.sync
                def _(sync):
                    sync.dma_start(out=x_t[:, :], in_=xb[:, :]).then_inc(
                        dma_a, 16
                    )
                    sync.wait_ge(dve_sem, 1)
                    sync.dma_start(out=out[:, :], in_=o_t[:, :]).then_inc(
                        dma_a, 16
                    )

                @block.scalar
                def _(scalar):
                    scalar.dma_start(out=wb_t[:, :], in_=wb[:, :]).then_inc(
                        dma_b, 16
                    )

                @block.tensor
                def _(tensor):
                    tensor.wait_ge(dma_b, 16)
                    tensor.wait_ge(dma_a, 16)
                    nc.tensor.matmul(
                        p_a[:, :], w1T_v, x_t[:, :], start=True, stop=True
                    ).then_inc(pe_sem, 1)
                    nc.tensor.matmul(
                        p_v[:, :], wdT_v, x_t[:, :], start=True, stop=True
                    ).then_inc(pe_sem, 1)

                @block.vector
                def _(vector):
                    # f32 copy of -b (the add-op scalar2 must be f32); the
                    # post-reduce drain covers this same-engine RAW too.
                    vector.wait_ge(dma_b, 16)
                    nc.vector.tensor_copy(negb[:, :], negb_v)
                    vector.wait_ge(pe_sem, 1)
                    nc.vector.reduce_max(
                        out=amax[:, :], in_=p_a[:, :],
                        axis=mybir.AxisListType.X,
                    )
                    # DVE pipeline is deep: same-engine RAW needs a drain.
                    nc.vector.drain()
                    vector.wait_ge(pe_sem, 2)
                    # q = (V + amax) max (-b)
                    nc.vector.tensor_scalar(
                        out=o_t[:, :],
                        in0=p_v[:, :],
                        scalar1=amax[:, :],
                        scalar2=negb[:, :],
                        op0=mybir.AluOpType.add,
                        op1=mybir.AluOpType.max,
                    ).then_inc(dve_sem, 1)
    finally:
        bass_mod.Bass.all_engine_barrier = orig_barrier

    nc.finalize()
    return nc


def _in_maps(x, W1, W2, b):
    bf = ml_dtypes.bfloat16
    x = np.asarray(x, dtype=np.float32)
    W1 = np.asarray(W1, dtype=np.float32)
    W2 = np.asarray(W2, dtype=np.float32)
    b = np.asarray(b, dtype=np.float32)
    pad = np.zeros((D, D - 1), dtype=np.float32)
    wb = np.ascontiguousarray(
        np.concatenate([W1.T, (W2 - W1).T, -b[:, None], pad], axis=1)
    ).astype(bf)
    xs = [
        np.ascontiguousarray(x[c % B]).astype(bf) for c in range(N_CORES)
    ]
    return [{"xb": xs[c], "wb": wb} for c in range(N_CORES)]


def kernel_raw(x, W1, W2, b, **run_kwargs):
    """Run the SPMD kernel; returns (full_output, BassKernelResults)."""
    global _NC_CACHE
    if _NC_CACHE is None:
        _NC_CACHE = _build()
    res = run_bass_kernel_spmd(
        _NC_CACHE, _in_maps(x, W1, W2, b), core_ids=list(range(N_CORES)),
        **run_kwargs,
    )
    b32 = np.asarray(b, dtype=np.float32)
    # device returns q = max(V + amax, -b); out = q + b
    out = np.stack(
        [
            res.results[c]["out"].astype(np.float32) + b32[:, None]
            for c in range(B)
        ],
        axis=0,
    )
    return out, res


def kernel(x, W1, W2, b):
    return kernel_raw(x, W1, W2, b)[0]
